# revision 27
# baseline (speedup 1.0000x reference)
"""Trainium2 Bass kernel for the segment-reduce cosine loss problem.

Reference computation (per sample b, S=32 labels):
  onehot[l,s] = (attributes[b,l] == s+1)
  seg_sum[s,:] = sum_l onehot[l,s] * text_feats[b,l,:]
  seg_mean     = seg_sum / count[s]
  cos[s] = <Vgs[b,s], seg_mean[s]> / max(|Vgs[b,s]| * |seg_mean[s]|, 1e-8)
  loss = mean_b (1 - mean_s cos[b,s]) = 1 - (sum_{b,s} cos) / (B*S)

Sharding: pure data parallel over batch; each of the 8 cores handles 8
samples.  The device does all the O(L*D) segment-sum work and returns
per (sample, attribute) num = <ss, vg> and nss = |ss|^2 partial sums --
except for the last eight d-tiles to land, whose segment-sum planes
ship raw [128, 8, S] for host-side O(S*D) reduction.  |Vg|^2 is
computed on the host from the original f32 Vgs.  The host gather step
finishes cos = num / sqrt(nss * nvg) and the mean over the 2048 values
(cosine is scale-invariant, so sums stand in for means).

Performance design (cost model: DMA 360 GB/s serialized on one device,
625 ns HWDGE per DMA, fp8 DoubleRow matmul 0.5 cyc/row at 2.4 GHz,
~300 ns per cross-engine dependency hop):
  - text_feats is quantized to fp8-e4m3 on the host, cutting the dominant
    HBM stream from 32 MB to 8 MB per core (~24 us at 360 GB/s); fp8
    noise lands ~3e-5 relative on the loss (gate is 2e-2).
  - Full samples are packed partition-major (8448 B per partition
    including the sample's transposed Vgs block) so each is a single
    128-descriptor DMA at full bus width.
  - Segment sums run on the PE in fp8 DoubleRow mode: lhsT = text d-tile
    [128L, 2, 128D] (stationary), rhs = onehot pair [128L, 2, 32]
    (moving), K=256 per instruction -> ssT [128D, 32S] per d-tile in
    PSUM, 16 cycles per matmul.  The d-tile loop is OUTER (4 consecutive
    matmuls per PSUM chain) so each bank drains right away and PSUM WAR
    dependencies never cascade into the tail.
  - The transposed [D, S] layout puts the epilogue on all 128 partitions:
    DVE computes prod=ssT*vgT and squares from SBUF, ACT shares PSUM
    drains, and the (num|nss) reductions are a ones-matmul chain.
  - The last THREE samples stream as d-tile pairs (728 ns DMAs) so the
    end of the stream deposits an even trickle of small engine ops; the
    tail samples' Vgs blocks ride in the early attributes DMA.  Sample
    7's first six d-tiles arrive EARLY (right after attributes) so its
    reduce chain closes at t5 long before the tail.
  - Tail latency is hop-count-bound, so the last eight d-tiles skip the
    whole epilogue: each segment-sum plane is copied PSUM->SBUF (one op,
    alternating DVE/ACT) and shipped raw (8 x S bf16 = 512 B/partition,
    full descriptor rate); the host computes their num/nss terms.
    Post-arrival chain: 4 matmuls -> one copy -> output DMA.  The
    (num|nss) block goes out in a separate earlier DMA.
  - The Tile scheduler's internal DMA model mispredicts arrival, so
    every unit is wrapped in tile_wait_until() with true
    cost-model-derived ready times; emission follows data-arrival order
    so the in-order queues never head-of-line block the tail.
"""

import numpy as np
import ml_dtypes

import concourse.mybir as mybir
import concourse.tile as tile
from concourse import bacc
from concourse.bass_utils import run_bass_kernel_spmd

B, L, D, S = 64, 1024, 1024, 32
N_CORES = 8
BPC = B // N_CORES        # samples per core
NCHUNK = L // 128         # L-chunks of 128 positions
NPAIR = NCHUNK // 2       # DoubleRow chunk pairs (256 positions each)
NDT = D // 128            # d-tiles of 128 feature columns
EPS = 1e-8
TXT_B = NPAIR * 2 * D     # 8192 text bytes per partition per sample
ROW_B = TXT_B + NDT * S   # + 256 transposed-Vgs bytes
NTAIL = 3                 # samples streamed as d-tile pairs (5, 6, 7)
NFULL = BPC - NTAIL       # samples fetched as one DMA (0..4)
TILE_B = NPAIR * 2 * 128  # 1024 text bytes per partition per d-tile (tails)
# tail vgt blocks carry only the d-tiles whose prod runs on device:
# b5 tiles 0..7, b6 tiles 0..1, b7 tiles 0..5
VGT_TILES = (NDT, 2, NDT - 2)
VGT_OFF = (0, NDT * S, (NDT + 2) * S)
ATTR_W = BPC * NCHUNK + sum(VGT_TILES) * S
NRAW = 8                  # raw ss planes: b6 t2..t7, b7 t6..t7 (512 B/part)

F32 = mybir.dt.float32
F8 = mybir.dt.float8e4
BF16 = mybir.dt.bfloat16
I8 = mybir.dt.int8
ALU = mybir.AluOpType
ACTF = mybir.ActivationFunctionType
PERF = mybir.MatmulPerfMode

NP_F8 = ml_dtypes.float8_e4m3


def build_bass():
    nc = bacc.Bacc(
        "TRN2", target_bir_lowering=False, debug=False, num_devices=N_CORES
    )
    # attributes block also carries the tail samples' transposed Vgs
    # (bitcast to fp8 on device) so they arrive in one early DMA
    attrs_d = nc.dram_tensor("attributes", [128, ATTR_W], I8, kind="ExternalInput")
    text_d = nc.dram_tensor("text_feats", [BPC, 128, ROW_B], F8, kind="ExternalInput")
    out_d = nc.dram_tensor("out", [1, BPC * 2 * S], F32, kind="ExternalOutput")
    raw_d = nc.dram_tensor("raw", [128, NRAW * S], BF16, kind="ExternalOutput")

    TAIL = tuple(range(BPC - NTAIL, BPC))
    b5, b6, b7 = TAIL

    with tile.TileContext(nc) as tc:
        with (
            tc.tile_pool(name="const", bufs=1) as const_pool,
            tc.tile_pool(name="text", bufs=NFULL) as text_pool,
            tc.tile_pool(name="oh", bufs=3) as oh_pool,
            tc.tile_pool(name="sst", bufs=2) as sst_pool,
            tc.tile_pool(name="combo", bufs=BPC) as combo_pool,
            tc.tile_pool(name="psum", bufs=6, space="PSUM") as psum_pool,
            tc.tile_pool(name="psumr", bufs=1, space="PSUM") as psumr_pool,
            tc.tile_pool(name="psumr7", bufs=1, space="PSUM") as psumr7_pool,
        ):
            # ---- constants ----
            iota_s = const_pool.tile([128, S], I8, name="iota_s")
            nc.gpsimd.iota(
                iota_s[:], pattern=[[1, S]], base=1, channel_multiplier=0,
                allow_small_or_imprecise_dtypes=True,
            )
            ones_bf = const_pool.tile([128, 1], BF16, name="ones_bf")
            nc.vector.memset(ones_bf[:], 1.0)

            # ---- DMA schedule ----
            # stream order: tx0, attrs, b7 tiles 0..5 (pairs), tx1..tx4,
            # b5 pairs, b6 pairs, b7 tile 6, b7 tile 7.  One SBUF tile PER
            # PAIR DMA for the tails (dependency tracking can be
            # tile-granular).  `arrive` mirrors the cost model's serial
            # transfer schedule so the Tile scheduler can be given true
            # ready times (its own DMA model mispredicts, which otherwise
            # reorders the tail queues).
            HEAD_NS = 1966
            TX_NS, ATTR_NS, PAIR_NS, SNG_NS = 3004, 296, 728, 364
            arrival = {}
            _cursor = [HEAD_NS]

            def arrive(key, dur):
                _cursor[0] += dur
                arrival[key] = _cursor[0]

            txs = [None] * BPC
            txp = {b: [None] * (NDT // 2 + 2) for b in TAIL}

            def tail_pair_dma(b, tp):
                tp_t = text_pool.tile(
                    [128, 2 * TILE_B], F8, tag=f"tx{b}", bufs=NDT // 2 + 2,
                    name=f"tx_{b}_{tp}",
                )
                txp[b][tp] = tp_t
                nc.sync.dma_start(
                    tp_t[:], text_d[b, :, tp * 2 * TILE_B:(tp + 1) * 2 * TILE_B]
                )
                arrive(("p", b, tp), PAIR_NS)

            txs[0] = text_pool.tile([128, ROW_B], F8, tag="tx", name="tx_0")
            nc.sync.dma_start(txs[0][:], text_d[0])
            arrival[("f", 0)] = HEAD_NS + TX_NS
            attr_sb = const_pool.tile([128, ATTR_W], I8, name="attr_sb")
            nc.sync.dma_start(attr_sb[:], attrs_d[:])
            _cursor[0] = HEAD_NS + TX_NS
            arrive("attr", ATTR_NS)
            # b7 d-tile pairs 0..2 arrive early (tile-major packing)
            for tp in range(3):
                tail_pair_dma(b7, tp)
            for b in range(1, NFULL):
                txs[b] = text_pool.tile([128, ROW_B], F8, tag="tx", name=f"tx_{b}")
                nc.sync.dma_start(txs[b][:], text_d[b])
                arrive(("f", b), TX_NS)
            for b in (b5, b6):
                for tp in range(NDT // 2):
                    tail_pair_dma(b, tp)
            # the closing single-tile DMAs of sample 7
            for k, t in enumerate((NDT - 2, NDT - 1)):
                st = text_pool.tile(
                    [128, TILE_B], F8, tag=f"tx{b7}", bufs=NDT // 2 + 2,
                    name=f"tx_{b7}_s{t}",
                )
                txp[b7][NDT // 2 + k] = st
                nc.sync.dma_start(
                    st[:], text_d[b7, :, t * TILE_B:(t + 1) * TILE_B]
                )
                arrive(("s", b7, t), SNG_NS)

            def at_ns(ns):
                # schedule hint: treat wrapped instructions as becoming
                # ready at `ns` (DMA-arrival-derived) in the scheduler
                return tc.tile_wait_until(ns / 1e6)

            # per-sample results: (num | nss), finished on the host;
            # raw ss planes for the last six d-tiles to land
            asm = const_pool.tile([1, BPC, 2 * S], F32, name="asm")
            raw = const_pool.tile([128, NRAW, S], BF16, name="raw")

            combos = []
            for b in range(BPC):
                cb = combo_pool.tile([128, NDT, 2 * S], BF16, tag="cb", name=f"cb_{b}")
                combos.append(cb)

            def vg_view(b):
                if b in TAIL:
                    i = b - TAIL[0]
                    lo = BPC * NCHUNK + VGT_OFF[i]
                    return attr_sb[:, lo:lo + VGT_TILES[i] * S].bitcast(
                        F8
                    ).rearrange("p (t s) -> p t s", s=S)
                return txs[b][:, TXT_B:ROW_B].rearrange("p (t s) -> p t s", s=S)

            def onehot(b, tag="oh", bufs=None):
                kw = {} if bufs is None else {"bufs": bufs}
                oh_all = oh_pool.tile([128, NCHUNK * S], F8, tag=tag,
                                      name=f"oh_{b}", **kw)
                nc.vector.tensor_tensor(
                    oh_all[:].rearrange("p (c s) -> p c s", s=S),
                    attr_sb[:, b * NCHUNK:(b + 1) * NCHUNK]
                    .unsqueeze(2).broadcast_to([128, NCHUNK, S]),
                    iota_s[:].unsqueeze(1).broadcast_to([128, NCHUNK, S]),
                    op=ALU.is_equal,
                )
                return oh_all[:].rearrange("p (c s) -> p c s", s=S)

            def full_sample(b):
                tx = txs[b]
                vg_v = vg_view(b)
                cb = combos[b]
                # combo[b]: [128, t, (prod | ss^2)]
                oh_v = onehot(b)

                # pair-major packing: [p, c, i, d]; d-tile outer so each
                # PSUM chain is 4 consecutive matmuls and drains at once
                tx_v = tx[:, 0:TXT_B].rearrange(
                    "p (c i d) -> p c i d", c=NPAIR, i=2
                )
                sst = sst_pool.tile([128, NDT, S], BF16, tag="sst", name=f"sst_{b}")
                for t in range(NDT):
                    pst = psum_pool.tile(
                        [128, S], F32, tag="ss", name=f"pst_{b}_{t}"
                    )
                    for c in range(NPAIR):
                        nc.tensor.matmul(
                            pst[:],
                            tx_v[:, c, :, t * 128:(t + 1) * 128],
                            oh_v[:, 2 * c:2 * c + 2, :],
                            start=(c == 0), stop=(c == NPAIR - 1),
                            perf_mode=PERF.DoubleRow,
                        )
                    # drain the bank (split the copies between DVE and ACT)
                    if t % 2 == 0:
                        nc.vector.tensor_copy(sst[:, t, :], pst[:])
                    else:
                        nc.scalar.activation(sst[:, t, :], pst[:], ACTF.Copy)
                # batched [128, 256] prod and ss^2 on DVE (2x for bf16)
                nc.vector.tensor_tensor(cb[:, :, 0:S], sst[:], vg_v, op=ALU.mult)
                nc.scalar.activation(cb[:, :, S:2 * S], sst[:], ACTF.Square)

                # partition-reduce (num | ss^2) over d via ones-matmul
                red = psumr_pool.tile([1, 2 * S], F32, tag="red", name=f"red_{b}")
                for t in range(NDT):
                    nc.tensor.matmul(
                        red[:], ones_bf[:], cb[:, t, :],
                        start=(t == 0), stop=(t == NDT - 1),
                    )
                if b % 2 == 0:
                    nc.vector.tensor_copy(asm[:, b, :], red[:])
                else:
                    nc.scalar.activation(asm[:, b, :], red[:], ACTF.Copy)

            # ---- tail helpers ----
            def tail_tile_mm(src, t2, b):
                src_v = src[:].rearrange(
                    "p (t c i e) -> p t c i e", t=src.shape[1] // TILE_B,
                    c=NPAIR, i=2,
                )
                pst = psum_pool.tile([128, S], F32, tag="ss", name=f"pst{b}_{t2}")
                for c in range(NPAIR):
                    nc.tensor.matmul(
                        pst[:],
                        src_v[:, t2, c, :, :],
                        oh_tail[b][:, 2 * c:2 * c + 2, :],
                        start=(c == 0), stop=(c == NPAIR - 1),
                        perf_mode=PERF.DoubleRow,
                    )
                return pst

            def tail_pair(b, tp):
                # drain the pair's banks (one ACT copy, one DVE copy), then
                # batched pair prod and square on DVE from SBUF (PSUM-direct
                # squaring is illegal - only one PSUM input per op)
                sst = sst_pool.tile(
                    [128, 2, S], BF16, tag=f"sstt{b}", bufs=NDT // 2,
                    name=f"sstt_{b}_{tp}",
                )
                for t2 in range(2):
                    pst = tail_tile_mm(txp[b][tp], t2, b)
                    if t2 == 0:
                        nc.scalar.activation(sst[:, t2, :], pst[:], ACTF.Copy)
                    else:
                        nc.vector.tensor_copy(sst[:, t2, :], pst[:])
                tsl = slice(2 * tp, 2 * tp + 2)
                nc.vector.tensor_tensor(
                    combos[b][:, tsl, 0:S], sst[:],
                    vg_view(b)[:, tsl, :], op=ALU.mult,
                )
                nc.vector.tensor_tensor(
                    combos[b][:, tsl, S:2 * S], sst[:], sst[:], op=ALU.mult
                )

            def raw_tile(src, t2, b, k, dve):
                # critical-path tile: segment-sum plane copied PSUM->SBUF
                # in ONE op and shipped raw; the host does num/nss for it
                pst = tail_tile_mm(src, t2, b)
                if dve:
                    nc.vector.tensor_copy(raw[:, k, :], pst[:])
                else:
                    nc.scalar.activation(raw[:, k, :], pst[:], ACTF.Copy)

            # ---- emission in data-arrival order, with schedule hints ----
            with at_ns(arrival[("f", 0)] + 900):
                full_sample(0)

            # tail early work: onehots
            with at_ns(arrival["attr"] + 900):
                oh_tail = {
                    b: onehot(b, tag=f"oht{b}", bufs=1) for b in TAIL
                }

            # b7 tiles 0..5 (early pairs) and its reduce, closed at t5
            for tp in range(3):
                with at_ns(arrival[("p", b7, tp)] + 900):
                    tail_pair(b7, tp)
            with at_ns(arrival[("p", b7, 2)] + 2600):
                red7 = psumr7_pool.tile([1, 2 * S], F32, tag="red7", name="red7")
                for t in range(NDT - 2):
                    nc.tensor.matmul(
                        red7[:], ones_bf[:], combos[b7][:, t, 0:2 * S],
                        start=(t == 0), stop=(t == NDT - 3),
                    )
                nc.vector.tensor_copy(asm[:, b7, 0:2 * S], red7[:])

            for b in range(1, NFULL):
                with at_ns(arrival[("f", b)] + 900):
                    full_sample(b)

            # b5 pairs + full reduce
            for tp in range(NDT // 2):
                with at_ns(arrival[("p", b5, tp)] + 900):
                    tail_pair(b5, tp)
            with at_ns(arrival[("p", b5, 3)] + 2600):
                red5 = psumr_pool.tile([1, 2 * S], F32, tag="red", name="red5")
                for t in range(NDT):
                    nc.tensor.matmul(
                        red5[:], ones_bf[:], combos[b5][:, t, 0:2 * S],
                        start=(t == 0), stop=(t == NDT - 1),
                    )
                nc.scalar.activation(asm[:, b5, 0:2 * S], red5[:], ACTF.Copy)

            # b6 pair 0; its reduce closes at t1 (tiles 2..7 go raw)
            with at_ns(arrival[("p", b6, 0)] + 900):
                tail_pair(b6, 0)
            with at_ns(arrival[("p", b6, 0)] + 2600):
                red6 = psumr_pool.tile([1, 2 * S], F32, tag="red", name="red6")
                for t in range(2):
                    nc.tensor.matmul(
                        red6[:], ones_bf[:], combos[b6][:, t, 0:2 * S],
                        start=(t == 0), stop=(t == 1),
                    )
                nc.scalar.activation(asm[:, b6, 0:2 * S], red6[:], ACTF.Copy)

            # (num | nss) block: gated only on mid-stream closes
            with at_ns(arrival[("p", b6, 0)] + 3300):
                nc.sync.dma_start(out_d[:], asm[:].rearrange("o b s -> o (b s)"))

            # ---- the tail: eight raw ss planes, then the raw DMA ----
            for tp in (1, 2, 3):
                with at_ns(arrival[("p", b6, tp)] + 900):
                    raw_tile(txp[b6][tp], 0, b6, 2 * (tp - 1), dve=True)
                    raw_tile(txp[b6][tp], 1, b6, 2 * (tp - 1) + 1, dve=False)
            with at_ns(arrival[("s", b7, NDT - 2)] + 900):
                raw_tile(txp[b7][NDT // 2], 0, b7, 6, dve=False)
            with at_ns(arrival[("s", b7, NDT - 1)] + 900):
                raw_tile(txp[b7][NDT // 2 + 1], 0, b7, 7, dve=True)
            with at_ns(arrival[("s", b7, NDT - 1)] + 1700):
                nc.sync.dma_start(raw_d[:], raw[:].rearrange("p k s -> p (k s)"))

    nc.compile()
    return nc


def pack_shard(attributes, text_feats, Vgs):
    """Host-side packing of one core's shard into the kernel's dram layout."""
    at = np.asarray(attributes)
    # attr[p, b, c] = attributes[b, c*128 + p], int8 (values 0..32),
    # followed by the tail samples' transposed Vgs blocks (fp8 bytes)
    attr_tp = np.empty((128, ATTR_W), dtype=np.int8)
    attr_tp[:, 0:BPC * NCHUNK] = (
        at.reshape(BPC, NCHUNK, 128).transpose(2, 0, 1)
        .reshape(128, BPC * NCHUNK).astype(np.int8)
    )

    tf8 = np.asarray(text_feats, dtype=np.float32).astype(NP_F8)
    vg8 = np.asarray(Vgs, dtype=np.float32).astype(NP_F8)
    t8 = np.empty((BPC, 128, ROW_B), dtype=NP_F8)
    x = tf8.reshape(BPC, NPAIR, 2, 128, D)
    for b in range(NFULL):
        # [p, c, i, d]
        t8[b, :, 0:TXT_B] = x[b].transpose(2, 0, 1, 3).reshape(128, TXT_B)
    for b in range(NFULL, BPC):
        # tail samples: d-tile-major [p, t, c, i, e]
        xb = x[b].reshape(NPAIR, 2, 128, NDT, 128)
        t8[b, :, 0:TXT_B] = xb.transpose(2, 3, 0, 1, 4).reshape(128, TXT_B)
    # vgt: [p, t, s] = Vgs[b, s, t*128+p]; fulls inline, tails via attrs
    vgt = vg8.reshape(BPC, S, NDT, 128).transpose(0, 3, 2, 1)
    t8[:, :, TXT_B:ROW_B] = vgt.reshape(BPC, 128, NDT * S)
    for i, b in enumerate(range(NFULL, BPC)):
        lo = BPC * NCHUNK + i * NDT * S
        attr_tp[:, lo:lo + NDT * S] = (
            vgt[b].reshape(128, NDT * S).view(np.int8)
        )
    return {"attributes": attr_tp, "text_feats": t8}


_NC_CACHE = None


def _get_nc():
    global _NC_CACHE
    if _NC_CACHE is None:
        _NC_CACHE = build_bass()
    return _NC_CACHE


# raw plane k -> (sample index, d-tile)
RAW_MAP = [(BPC - 2, 2), (BPC - 2, 3), (BPC - 2, 4), (BPC - 2, 5),
           (BPC - 2, 6), (BPC - 2, 7), (BPC - 1, 6), (BPC - 1, 7)]


def _finish(out_flat, raw_flat, Vgs_shard):
    """Host finish for one core: fold the raw ss planes into (num, nss),
    compute nvg from f32 Vgs, then cos = num / sqrt(nss * nvg), summed."""
    arr = np.asarray(out_flat, dtype=np.float64).reshape(BPC, 2, S)
    num, nss = arr[:, 0, :].copy(), arr[:, 1, :].copy()
    vg = np.asarray(Vgs_shard, dtype=np.float64)       # [BPC, S, D]
    rw = np.asarray(raw_flat, dtype=np.float64).reshape(128, NRAW, S)
    for k, (b, t) in enumerate(RAW_MAP):
        ss = rw[:, k, :]                               # [128p, S] = ss[d,s]
        vg_t = vg[b, :, t * 128:(t + 1) * 128].T       # [128p, S]
        num[b] += (ss * vg_t).sum(axis=0)
        nss[b] += (ss * ss).sum(axis=0)
    nvg = (vg * vg).sum(axis=2)                        # [BPC, S]
    den = np.maximum(np.sqrt(nss * nvg), EPS)
    return float((num / den).sum())


def kernel(attributes: np.ndarray, text_feats: np.ndarray, Vgs: np.ndarray) -> np.ndarray:
    assert attributes.shape == (B, L) and attributes.dtype == np.int32
    assert text_feats.shape == (B, L, D)
    assert Vgs.shape == (B, S, D)
    nc = _get_nc()
    in_maps = [
        pack_shard(
            attributes[i * BPC:(i + 1) * BPC],
            text_feats[i * BPC:(i + 1) * BPC],
            Vgs[i * BPC:(i + 1) * BPC],
        )
        for i in range(N_CORES)
    ]
    res = run_bass_kernel_spmd(nc, in_maps, core_ids=list(range(N_CORES)))
    total = sum(
        _finish(r["out"], r["raw"], Vgs[i * BPC:(i + 1) * BPC])
        for i, r in enumerate(res.results)
    )
    loss = 1.0 - total / (B * S)
    return np.asarray(loss, dtype=np.float32)


# revision 29
# speedup vs baseline: 1.0030x; 1.0030x over previous
"""Trainium2 Bass kernel for the segment-reduce cosine loss problem.

Reference computation (per sample b, S=32 labels):
  onehot[l,s] = (attributes[b,l] == s+1)
  seg_sum[s,:] = sum_l onehot[l,s] * text_feats[b,l,:]
  seg_mean     = seg_sum / count[s]
  cos[s] = <Vgs[b,s], seg_mean[s]> / max(|Vgs[b,s]| * |seg_mean[s]|, 1e-8)
  loss = mean_b (1 - mean_s cos[b,s]) = 1 - (sum_{b,s} cos) / (B*S)

Sharding: pure data parallel over batch; each of the 8 cores handles 8
samples.  The device does all the O(L*D) segment-sum work and returns
per (sample, attribute) num = <ss, vg> and nss = |ss|^2 partial sums --
except for the last eight d-tiles to land, whose segment-sum planes
ship raw [128, 8, S] for host-side O(S*D) reduction.  |Vg|^2 is
computed on the host from the original f32 Vgs.  The host gather step
finishes cos = num / sqrt(nss * nvg) and the mean over the 2048 values
(cosine is scale-invariant, so sums stand in for means).

Performance design (cost model: DMA 360 GB/s serialized on one device,
625 ns HWDGE per DMA, fp8 DoubleRow matmul 0.5 cyc/row at 2.4 GHz,
~300 ns per cross-engine dependency hop):
  - text_feats is quantized to fp8-e4m3 on the host, cutting the dominant
    HBM stream from 32 MB to 8 MB per core (~24 us at 360 GB/s); fp8
    noise lands ~3e-5 relative on the loss (gate is 2e-2).
  - Full samples are packed partition-major (8448 B per partition
    including the sample's transposed Vgs block) so each is a single
    128-descriptor DMA at full bus width.
  - Segment sums run on the PE in fp8 DoubleRow mode: lhsT = text d-tile
    [128L, 2, 128D] (stationary), rhs = onehot pair [128L, 2, 32]
    (moving), K=256 per instruction -> ssT [128D, 32S] per d-tile in
    PSUM, 16 cycles per matmul.  The d-tile loop is OUTER (4 consecutive
    matmuls per PSUM chain) so each bank drains right away and PSUM WAR
    dependencies never cascade into the tail.
  - The transposed [D, S] layout puts the epilogue on all 128 partitions:
    DVE computes prod=ssT*vgT and squares from SBUF, ACT shares PSUM
    drains, and the (num|nss) reductions are a ones-matmul chain.
  - The last THREE samples stream as d-tile pairs (728 ns DMAs) so the
    end of the stream deposits an even trickle of small engine ops; the
    tail samples' Vgs blocks ride in the early attributes DMA.  Sample
    7's first six d-tiles arrive EARLY (right after attributes) so its
    reduce chain closes at t5 long before the tail.
  - Tail latency is hop-count-bound, so the last eight d-tiles skip the
    whole epilogue: each segment-sum plane is copied PSUM->SBUF (one op,
    alternating DVE/ACT) and shipped raw (8 x S bf16 = 512 B/partition,
    full descriptor rate); the host computes their num/nss terms.
    Post-arrival chain: 4 matmuls -> one copy -> output DMA.  The
    (num|nss) block goes out in a separate earlier DMA.
  - The Tile scheduler's internal DMA model mispredicts arrival, so
    every unit is wrapped in tile_wait_until() with true
    cost-model-derived ready times; emission follows data-arrival order
    so the in-order queues never head-of-line block the tail.
"""

import numpy as np
import ml_dtypes

import concourse.mybir as mybir
import concourse.tile as tile
from concourse import bacc
from concourse.bass_utils import run_bass_kernel_spmd

B, L, D, S = 64, 1024, 1024, 32
N_CORES = 8
BPC = B // N_CORES        # samples per core
NCHUNK = L // 128         # L-chunks of 128 positions
NPAIR = NCHUNK // 2       # DoubleRow chunk pairs (256 positions each)
NDT = D // 128            # d-tiles of 128 feature columns
EPS = 1e-8
TXT_B = NPAIR * 2 * D     # 8192 text bytes per partition per sample
ROW_B = TXT_B + NDT * S   # + 256 transposed-Vgs bytes
NTAIL = 3                 # samples streamed as d-tile pairs (5, 6, 7)
NFULL = BPC - NTAIL       # samples fetched as one DMA (0..4)
TILE_B = NPAIR * 2 * 128  # 1024 text bytes per partition per d-tile (tails)
# tail vgt blocks carry only the d-tiles whose prod runs on device:
# b5 tiles 0..7, b6 tiles 0..1, b7 tiles 0..5
VGT_TILES = (NDT, 2, NDT - 2)
VGT_OFF = (0, NDT * S, (NDT + 2) * S)
ATTR_W = BPC * NCHUNK + sum(VGT_TILES) * S
NRAW = 8                  # raw ss planes: b6 t2..t7, b7 t6..t7 (512 B/part)

F32 = mybir.dt.float32
F8 = mybir.dt.float8e4
BF16 = mybir.dt.bfloat16
I8 = mybir.dt.int8
ALU = mybir.AluOpType
ACTF = mybir.ActivationFunctionType
PERF = mybir.MatmulPerfMode

NP_F8 = ml_dtypes.float8_e4m3


def build_bass():
    nc = bacc.Bacc(
        "TRN2", target_bir_lowering=False, debug=False, num_devices=N_CORES
    )
    # attributes block also carries the tail samples' transposed Vgs
    # (bitcast to fp8 on device) so they arrive in one early DMA
    attrs_d = nc.dram_tensor("attributes", [128, ATTR_W], I8, kind="ExternalInput")
    text_d = nc.dram_tensor("text_feats", [BPC, 128, ROW_B], F8, kind="ExternalInput")
    out_d = nc.dram_tensor("out", [1, BPC * 2 * S], F32, kind="ExternalOutput")
    raw_d = nc.dram_tensor("raw", [128, NRAW * S], BF16, kind="ExternalOutput")

    TAIL = tuple(range(BPC - NTAIL, BPC))
    b5, b6, b7 = TAIL

    with tile.TileContext(nc) as tc:
        with (
            tc.tile_pool(name="const", bufs=1) as const_pool,
            tc.tile_pool(name="text", bufs=NFULL) as text_pool,
            tc.tile_pool(name="oh", bufs=3) as oh_pool,
            tc.tile_pool(name="sst", bufs=2) as sst_pool,
            tc.tile_pool(name="combo", bufs=BPC) as combo_pool,
            tc.tile_pool(name="psum", bufs=6, space="PSUM") as psum_pool,
            tc.tile_pool(name="psumr", bufs=1, space="PSUM") as psumr_pool,
            tc.tile_pool(name="psumr7", bufs=1, space="PSUM") as psumr7_pool,
        ):
            # ---- constants ----
            iota_s = const_pool.tile([128, S], I8, name="iota_s")
            nc.gpsimd.iota(
                iota_s[:], pattern=[[1, S]], base=1, channel_multiplier=0,
                allow_small_or_imprecise_dtypes=True,
            )
            ones_bf = const_pool.tile([128, 1], BF16, name="ones_bf")
            nc.vector.memset(ones_bf[:], 1.0)

            # ---- DMA schedule ----
            # stream order: tx0, attrs, b7 tiles 0..5 (pairs), tx1..tx4,
            # b5 pairs, b6 pairs, b7 tile 6, b7 tile 7.  One SBUF tile PER
            # PAIR DMA for the tails (dependency tracking can be
            # tile-granular).  `arrive` mirrors the cost model's serial
            # transfer schedule so the Tile scheduler can be given true
            # ready times (its own DMA model mispredicts, which otherwise
            # reorders the tail queues).
            HEAD_NS = 1966
            TX_NS, ATTR_NS, PAIR_NS, SNG_NS = 3004, 205, 728, 364
            arrival = {}
            _cursor = [HEAD_NS]

            def arrive(key, dur):
                _cursor[0] += dur
                arrival[key] = _cursor[0]

            txs = [None] * BPC
            txp = {b: [None] * (NDT // 2 + 2) for b in TAIL}

            def tail_pair_dma(b, tp):
                tp_t = text_pool.tile(
                    [128, 2 * TILE_B], F8, tag=f"tx{b}", bufs=NDT // 2 + 2,
                    name=f"tx_{b}_{tp}",
                )
                txp[b][tp] = tp_t
                nc.sync.dma_start(
                    tp_t[:], text_d[b, :, tp * 2 * TILE_B:(tp + 1) * 2 * TILE_B]
                )
                arrive(("p", b, tp), PAIR_NS)

            txs[0] = text_pool.tile([128, ROW_B], F8, tag="tx", name="tx_0")
            nc.sync.dma_start(txs[0][:], text_d[0])
            arrival[("f", 0)] = HEAD_NS + TX_NS
            attr_sb = const_pool.tile([128, ATTR_W], I8, name="attr_sb")
            nc.sync.dma_start(attr_sb[:], attrs_d[:])
            _cursor[0] = HEAD_NS + TX_NS
            arrive("attr", ATTR_NS)
            # b7 d-tile pairs 0..2 arrive early (tile-major packing)
            for tp in range(3):
                tail_pair_dma(b7, tp)
            for b in range(1, NFULL):
                txs[b] = text_pool.tile([128, ROW_B], F8, tag="tx", name=f"tx_{b}")
                nc.sync.dma_start(txs[b][:], text_d[b])
                arrive(("f", b), TX_NS)
            for b in (b5, b6):
                for tp in range(NDT // 2):
                    tail_pair_dma(b, tp)
            # the closing single-tile DMAs of sample 7
            for k, t in enumerate((NDT - 2, NDT - 1)):
                st = text_pool.tile(
                    [128, TILE_B], F8, tag=f"tx{b7}", bufs=NDT // 2 + 2,
                    name=f"tx_{b7}_s{t}",
                )
                txp[b7][NDT // 2 + k] = st
                nc.sync.dma_start(
                    st[:], text_d[b7, :, t * TILE_B:(t + 1) * TILE_B]
                )
                arrive(("s", b7, t), SNG_NS)

            def at_ns(ns):
                # schedule hint: treat wrapped instructions as becoming
                # ready at `ns` (DMA-arrival-derived) in the scheduler
                return tc.tile_wait_until(ns / 1e6)

            # per-sample results: (num | nss), finished on the host;
            # raw ss planes for the last six d-tiles to land
            asm = const_pool.tile([1, BPC, 2 * S], F32, name="asm")
            raw = const_pool.tile([128, NRAW, S], BF16, name="raw")

            combos = []
            for b in range(BPC):
                cb = combo_pool.tile([128, NDT, 2 * S], BF16, tag="cb", name=f"cb_{b}")
                combos.append(cb)

            def vg_view(b):
                if b in TAIL:
                    i = b - TAIL[0]
                    lo = BPC * NCHUNK + VGT_OFF[i]
                    return attr_sb[:, lo:lo + VGT_TILES[i] * S].bitcast(
                        F8
                    ).rearrange("p (t s) -> p t s", s=S)
                return txs[b][:, TXT_B:ROW_B].rearrange("p (t s) -> p t s", s=S)

            def onehot(b, tag="oh", bufs=None):
                kw = {} if bufs is None else {"bufs": bufs}
                oh_all = oh_pool.tile([128, NCHUNK * S], F8, tag=tag,
                                      name=f"oh_{b}", **kw)
                nc.vector.tensor_tensor(
                    oh_all[:].rearrange("p (c s) -> p c s", s=S),
                    attr_sb[:, b * NCHUNK:(b + 1) * NCHUNK]
                    .unsqueeze(2).broadcast_to([128, NCHUNK, S]),
                    iota_s[:].unsqueeze(1).broadcast_to([128, NCHUNK, S]),
                    op=ALU.is_equal,
                )
                return oh_all[:].rearrange("p (c s) -> p c s", s=S)

            def full_sample(b):
                tx = txs[b]
                vg_v = vg_view(b)
                cb = combos[b]
                # combo[b]: [128, t, (prod | ss^2)]
                oh_v = onehot(b)

                # pair-major packing: [p, c, i, d]; d-tile outer so each
                # PSUM chain is 4 consecutive matmuls and drains at once
                tx_v = tx[:, 0:TXT_B].rearrange(
                    "p (c i d) -> p c i d", c=NPAIR, i=2
                )
                sst = sst_pool.tile([128, NDT, S], BF16, tag="sst", name=f"sst_{b}")
                for t in range(NDT):
                    pst = psum_pool.tile(
                        [128, S], F32, tag="ss", name=f"pst_{b}_{t}"
                    )
                    for c in range(NPAIR):
                        nc.tensor.matmul(
                            pst[:],
                            tx_v[:, c, :, t * 128:(t + 1) * 128],
                            oh_v[:, 2 * c:2 * c + 2, :],
                            start=(c == 0), stop=(c == NPAIR - 1),
                            perf_mode=PERF.DoubleRow,
                        )
                    # drain the bank (split the copies between DVE and ACT)
                    if t % 2 == 0:
                        nc.vector.tensor_copy(sst[:, t, :], pst[:])
                    else:
                        nc.scalar.activation(sst[:, t, :], pst[:], ACTF.Copy)
                # batched [128, 256] prod and ss^2 on DVE (2x for bf16)
                nc.vector.tensor_tensor(cb[:, :, 0:S], sst[:], vg_v, op=ALU.mult)
                nc.scalar.activation(cb[:, :, S:2 * S], sst[:], ACTF.Square)

                # partition-reduce (num | ss^2) over d via ones-matmul
                red = psumr_pool.tile([1, 2 * S], F32, tag="red", name=f"red_{b}")
                for t in range(NDT):
                    nc.tensor.matmul(
                        red[:], ones_bf[:], cb[:, t, :],
                        start=(t == 0), stop=(t == NDT - 1),
                    )
                if b % 2 == 0:
                    nc.vector.tensor_copy(asm[:, b, :], red[:])
                else:
                    nc.scalar.activation(asm[:, b, :], red[:], ACTF.Copy)

            # ---- tail helpers ----
            def tail_tile_mm(src, t2, b):
                src_v = src[:].rearrange(
                    "p (t c i e) -> p t c i e", t=src.shape[1] // TILE_B,
                    c=NPAIR, i=2,
                )
                pst = psum_pool.tile([128, S], F32, tag="ss", name=f"pst{b}_{t2}")
                for c in range(NPAIR):
                    nc.tensor.matmul(
                        pst[:],
                        src_v[:, t2, c, :, :],
                        oh_tail[b][:, 2 * c:2 * c + 2, :],
                        start=(c == 0), stop=(c == NPAIR - 1),
                        perf_mode=PERF.DoubleRow,
                    )
                return pst

            def tail_pair(b, tp):
                # drain the pair's banks (one ACT copy, one DVE copy), then
                # batched pair prod and square on DVE from SBUF (PSUM-direct
                # squaring is illegal - only one PSUM input per op)
                sst = sst_pool.tile(
                    [128, 2, S], BF16, tag=f"sstt{b}", bufs=NDT // 2,
                    name=f"sstt_{b}_{tp}",
                )
                for t2 in range(2):
                    pst = tail_tile_mm(txp[b][tp], t2, b)
                    if t2 == 0:
                        nc.scalar.activation(sst[:, t2, :], pst[:], ACTF.Copy)
                    else:
                        nc.vector.tensor_copy(sst[:, t2, :], pst[:])
                tsl = slice(2 * tp, 2 * tp + 2)
                nc.vector.tensor_tensor(
                    combos[b][:, tsl, 0:S], sst[:],
                    vg_view(b)[:, tsl, :], op=ALU.mult,
                )
                nc.vector.tensor_tensor(
                    combos[b][:, tsl, S:2 * S], sst[:], sst[:], op=ALU.mult
                )

            def raw_tile(src, t2, b, k, dve):
                # critical-path tile: segment-sum plane copied PSUM->SBUF
                # in ONE op and shipped raw; the host does num/nss for it
                pst = tail_tile_mm(src, t2, b)
                if dve:
                    nc.vector.tensor_copy(raw[:, k, :], pst[:])
                else:
                    nc.scalar.activation(raw[:, k, :], pst[:], ACTF.Copy)

            # ---- emission in data-arrival order, with schedule hints ----
            with at_ns(arrival[("f", 0)] + 900):
                full_sample(0)

            # tail early work: onehots
            with at_ns(arrival["attr"] + 900):
                oh_tail = {
                    b: onehot(b, tag=f"oht{b}", bufs=1) for b in TAIL
                }

            # b7 tiles 0..5 (early pairs) and its reduce, closed at t5
            for tp in range(3):
                with at_ns(arrival[("p", b7, tp)] + 900):
                    tail_pair(b7, tp)
            with at_ns(arrival[("p", b7, 2)] + 2600):
                red7 = psumr7_pool.tile([1, 2 * S], F32, tag="red7", name="red7")
                for t in range(NDT - 2):
                    nc.tensor.matmul(
                        red7[:], ones_bf[:], combos[b7][:, t, 0:2 * S],
                        start=(t == 0), stop=(t == NDT - 3),
                    )
                nc.vector.tensor_copy(asm[:, b7, 0:2 * S], red7[:])

            for b in range(1, NFULL):
                with at_ns(arrival[("f", b)] + 900):
                    full_sample(b)

            # b5 pairs + full reduce
            for tp in range(NDT // 2):
                with at_ns(arrival[("p", b5, tp)] + 900):
                    tail_pair(b5, tp)
            with at_ns(arrival[("p", b5, 3)] + 2600):
                red5 = psumr_pool.tile([1, 2 * S], F32, tag="red", name="red5")
                for t in range(NDT):
                    nc.tensor.matmul(
                        red5[:], ones_bf[:], combos[b5][:, t, 0:2 * S],
                        start=(t == 0), stop=(t == NDT - 1),
                    )
                nc.scalar.activation(asm[:, b5, 0:2 * S], red5[:], ACTF.Copy)

            # b6 pair 0; its reduce closes at t1 (tiles 2..7 go raw)
            with at_ns(arrival[("p", b6, 0)] + 900):
                tail_pair(b6, 0)
            with at_ns(arrival[("p", b6, 0)] + 2600):
                red6 = psumr_pool.tile([1, 2 * S], F32, tag="red", name="red6")
                for t in range(2):
                    nc.tensor.matmul(
                        red6[:], ones_bf[:], combos[b6][:, t, 0:2 * S],
                        start=(t == 0), stop=(t == 1),
                    )
                nc.scalar.activation(asm[:, b6, 0:2 * S], red6[:], ACTF.Copy)

            # (num | nss) block: gated only on mid-stream closes
            with at_ns(arrival[("p", b6, 0)] + 3300):
                nc.sync.dma_start(out_d[:], asm[:].rearrange("o b s -> o (b s)"))

            # ---- the tail: eight raw ss planes, then the raw DMA ----
            for tp in (1, 2, 3):
                with at_ns(arrival[("p", b6, tp)] + 900):
                    raw_tile(txp[b6][tp], 0, b6, 2 * (tp - 1), dve=True)
                    raw_tile(txp[b6][tp], 1, b6, 2 * (tp - 1) + 1, dve=False)
            with at_ns(arrival[("s", b7, NDT - 2)] + 900):
                raw_tile(txp[b7][NDT // 2], 0, b7, 6, dve=False)
            with at_ns(arrival[("s", b7, NDT - 1)] + 900):
                raw_tile(txp[b7][NDT // 2 + 1], 0, b7, 7, dve=True)
            with at_ns(arrival[("s", b7, NDT - 1)] + 1700):
                nc.sync.dma_start(raw_d[:], raw[:].rearrange("p k s -> p (k s)"))

    nc.compile()
    return nc


def pack_shard(attributes, text_feats, Vgs):
    """Host-side packing of one core's shard into the kernel's dram layout."""
    at = np.asarray(attributes)
    # attr[p, b, c] = attributes[b, c*128 + p], int8 (values 0..32),
    # followed by the tail samples' transposed Vgs blocks (fp8 bytes)
    attr_tp = np.empty((128, ATTR_W), dtype=np.int8)
    attr_tp[:, 0:BPC * NCHUNK] = (
        at.reshape(BPC, NCHUNK, 128).transpose(2, 0, 1)
        .reshape(128, BPC * NCHUNK).astype(np.int8)
    )

    tf8 = np.asarray(text_feats, dtype=np.float32).astype(NP_F8)
    vg8 = np.asarray(Vgs, dtype=np.float32).astype(NP_F8)
    t8 = np.empty((BPC, 128, ROW_B), dtype=NP_F8)
    x = tf8.reshape(BPC, NPAIR, 2, 128, D)
    for b in range(NFULL):
        # [p, c, i, d]
        t8[b, :, 0:TXT_B] = x[b].transpose(2, 0, 1, 3).reshape(128, TXT_B)
    for b in range(NFULL, BPC):
        # tail samples: d-tile-major [p, t, c, i, e]
        xb = x[b].reshape(NPAIR, 2, 128, NDT, 128)
        t8[b, :, 0:TXT_B] = xb.transpose(2, 3, 0, 1, 4).reshape(128, TXT_B)
    # vgt: [p, t, s] = Vgs[b, s, t*128+p]; fulls inline, tails via attrs
    # (tails carry only the d-tiles whose prod runs on device)
    vgt = vg8.reshape(BPC, S, NDT, 128).transpose(0, 3, 2, 1)
    t8[:, :, TXT_B:ROW_B] = vgt.reshape(BPC, 128, NDT * S)
    for i, b in enumerate(range(NFULL, BPC)):
        lo = BPC * NCHUNK + VGT_OFF[i]
        nt = VGT_TILES[i]
        attr_tp[:, lo:lo + nt * S] = (
            vgt[b, :, 0:nt, :].reshape(128, nt * S).view(np.int8)
        )
    return {"attributes": attr_tp, "text_feats": t8}


_NC_CACHE = None


def _get_nc():
    global _NC_CACHE
    if _NC_CACHE is None:
        _NC_CACHE = build_bass()
    return _NC_CACHE


# raw plane k -> (sample index, d-tile)
RAW_MAP = [(BPC - 2, 2), (BPC - 2, 3), (BPC - 2, 4), (BPC - 2, 5),
           (BPC - 2, 6), (BPC - 2, 7), (BPC - 1, 6), (BPC - 1, 7)]


def _finish(out_flat, raw_flat, Vgs_shard):
    """Host finish for one core: fold the raw ss planes into (num, nss),
    compute nvg from f32 Vgs, then cos = num / sqrt(nss * nvg), summed."""
    arr = np.asarray(out_flat, dtype=np.float64).reshape(BPC, 2, S)
    num, nss = arr[:, 0, :].copy(), arr[:, 1, :].copy()
    vg = np.asarray(Vgs_shard, dtype=np.float64)       # [BPC, S, D]
    rw = np.asarray(raw_flat, dtype=np.float64).reshape(128, NRAW, S)
    for k, (b, t) in enumerate(RAW_MAP):
        ss = rw[:, k, :]                               # [128p, S] = ss[d,s]
        vg_t = vg[b, :, t * 128:(t + 1) * 128].T       # [128p, S]
        num[b] += (ss * vg_t).sum(axis=0)
        nss[b] += (ss * ss).sum(axis=0)
    nvg = (vg * vg).sum(axis=2)                        # [BPC, S]
    den = np.maximum(np.sqrt(nss * nvg), EPS)
    return float((num / den).sum())


def kernel(attributes: np.ndarray, text_feats: np.ndarray, Vgs: np.ndarray) -> np.ndarray:
    assert attributes.shape == (B, L) and attributes.dtype == np.int32
    assert text_feats.shape == (B, L, D)
    assert Vgs.shape == (B, S, D)
    nc = _get_nc()
    in_maps = [
        pack_shard(
            attributes[i * BPC:(i + 1) * BPC],
            text_feats[i * BPC:(i + 1) * BPC],
            Vgs[i * BPC:(i + 1) * BPC],
        )
        for i in range(N_CORES)
    ]
    res = run_bass_kernel_spmd(nc, in_maps, core_ids=list(range(N_CORES)))
    total = sum(
        _finish(r["out"], r["raw"], Vgs[i * BPC:(i + 1) * BPC])
        for i, r in enumerate(res.results)
    )
    loss = 1.0 - total / (B * S)
    return np.asarray(loss, dtype=np.float32)


# revision 30
# speedup vs baseline: 1.0056x; 1.0026x over previous
"""Trainium2 Bass kernel for the segment-reduce cosine loss problem.

Reference computation (per sample b, S=32 labels):
  onehot[l,s] = (attributes[b,l] == s+1)
  seg_sum[s,:] = sum_l onehot[l,s] * text_feats[b,l,:]
  seg_mean     = seg_sum / count[s]
  cos[s] = <Vgs[b,s], seg_mean[s]> / max(|Vgs[b,s]| * |seg_mean[s]|, 1e-8)
  loss = mean_b (1 - mean_s cos[b,s]) = 1 - (sum_{b,s} cos) / (B*S)

Sharding: pure data parallel over batch; each of the 8 cores handles 8
samples.  The device does all the O(L*D) segment-sum work and returns
per (sample, attribute) num = <ss, vg> and nss = |ss|^2 partial sums --
except for the last eight d-tiles to land, whose segment-sum planes
ship raw [128, 8, S] for host-side O(S*D) reduction.  |Vg|^2 is
computed on the host from the original f32 Vgs.  The host gather step
finishes cos = num / sqrt(nss * nvg) and the mean over the 2048 values
(cosine is scale-invariant, so sums stand in for means).

Performance design (cost model: DMA 360 GB/s serialized on one device,
625 ns HWDGE per DMA, fp8 DoubleRow matmul 0.5 cyc/row at 2.4 GHz,
~300 ns per cross-engine dependency hop):
  - text_feats is quantized to fp8-e4m3 on the host, cutting the dominant
    HBM stream from 32 MB to 8 MB per core (~24 us at 360 GB/s); fp8
    noise lands ~3e-5 relative on the loss (gate is 2e-2).
  - Full samples are packed partition-major (8448 B per partition
    including the sample's transposed Vgs block) so each is a single
    128-descriptor DMA at full bus width.
  - Segment sums run on the PE in fp8 DoubleRow mode: lhsT = text d-tile
    [128L, 2, 128D] (stationary), rhs = onehot pair [128L, 2, 32]
    (moving), K=256 per instruction -> ssT [128D, 32S] per d-tile in
    PSUM, 16 cycles per matmul.  The d-tile loop is OUTER (4 consecutive
    matmuls per PSUM chain) so each bank drains right away and PSUM WAR
    dependencies never cascade into the tail.
  - The transposed [D, S] layout puts the epilogue on all 128 partitions:
    DVE computes prod=ssT*vgT and squares from SBUF, ACT shares PSUM
    drains, and the (num|nss) reductions are a ones-matmul chain.
  - The last THREE samples stream as d-tile pairs (728 ns DMAs) so the
    end of the stream deposits an even trickle of small engine ops; the
    tail samples' Vgs blocks ride in the early attributes DMA.  Sample
    7's first six d-tiles arrive EARLY (right after attributes) so its
    reduce chain closes at t5 long before the tail.
  - Tail latency is hop-count-bound, so the last eight d-tiles skip the
    whole epilogue: each segment-sum plane is copied PSUM->SBUF (one op,
    alternating DVE/ACT) and shipped raw (8 x S bf16 = 512 B/partition,
    full descriptor rate); the host computes their num/nss terms.
    Post-arrival chain: 4 matmuls -> one copy -> output DMA.  The
    (num|nss) block goes out in a separate earlier DMA.
  - The Tile scheduler's internal DMA model mispredicts arrival, so
    every unit is wrapped in tile_wait_until() with true
    cost-model-derived ready times; emission follows data-arrival order
    so the in-order queues never head-of-line block the tail.
"""

import numpy as np
import ml_dtypes

import concourse.mybir as mybir
import concourse.tile as tile
from concourse import bacc
from concourse.bass_utils import run_bass_kernel_spmd

B, L, D, S = 64, 1024, 1024, 32
N_CORES = 8
BPC = B // N_CORES        # samples per core
NCHUNK = L // 128         # L-chunks of 128 positions
NPAIR = NCHUNK // 2       # DoubleRow chunk pairs (256 positions each)
NDT = D // 128            # d-tiles of 128 feature columns
EPS = 1e-8
TXT_B = NPAIR * 2 * D     # 8192 text bytes per partition per sample
ROW_B = TXT_B + NDT * S   # + 256 transposed-Vgs bytes
NTAIL = 3                 # samples streamed as d-tile pairs (5, 6, 7)
NFULL = BPC - NTAIL       # samples fetched as one DMA (0..4)
TILE_B = NPAIR * 2 * 128  # 1024 text bytes per partition per d-tile (tails)
# tail vgt blocks carry only the d-tiles whose prod runs on device:
# b5 tiles 0..7, b6 tiles 0..1, b7 tiles 0..5
VGT_TILES = (NDT, 2, NDT - 2)
VGT_OFF = (0, NDT * S, (NDT + 2) * S)
ATTR_W = BPC * NCHUNK + sum(VGT_TILES) * S
NRAW = 8                  # raw ss planes: b6 t2..t7, b7 t6..t7 (512 B/part)

F32 = mybir.dt.float32
F8 = mybir.dt.float8e4
BF16 = mybir.dt.bfloat16
I8 = mybir.dt.int8
ALU = mybir.AluOpType
ACTF = mybir.ActivationFunctionType
PERF = mybir.MatmulPerfMode

NP_F8 = ml_dtypes.float8_e4m3


def build_bass():
    nc = bacc.Bacc(
        "TRN2", target_bir_lowering=False, debug=False, num_devices=N_CORES
    )
    # attributes block also carries the tail samples' transposed Vgs
    # (bitcast to fp8 on device) so they arrive in one early DMA
    attrs_d = nc.dram_tensor("attributes", [128, ATTR_W], I8, kind="ExternalInput")
    text_d = nc.dram_tensor("text_feats", [BPC, 128, ROW_B], F8, kind="ExternalInput")
    out_d = nc.dram_tensor("out", [1, BPC * 2 * S], F32, kind="ExternalOutput")
    raw_d = nc.dram_tensor("raw", [128, NRAW * S], BF16, kind="ExternalOutput")

    TAIL = tuple(range(BPC - NTAIL, BPC))
    b5, b6, b7 = TAIL

    with tile.TileContext(nc) as tc:
        with (
            tc.tile_pool(name="const", bufs=1) as const_pool,
            tc.tile_pool(name="text", bufs=NFULL) as text_pool,
            tc.tile_pool(name="oh", bufs=3) as oh_pool,
            tc.tile_pool(name="sst", bufs=2) as sst_pool,
            tc.tile_pool(name="combo", bufs=BPC) as combo_pool,
            tc.tile_pool(name="psum", bufs=6, space="PSUM") as psum_pool,
            tc.tile_pool(name="psumr", bufs=1, space="PSUM") as psumr_pool,
            tc.tile_pool(name="psumr7", bufs=1, space="PSUM") as psumr7_pool,
        ):
            # ---- constants ----
            iota_s = const_pool.tile([128, S], I8, name="iota_s")
            nc.gpsimd.iota(
                iota_s[:], pattern=[[1, S]], base=1, channel_multiplier=0,
                allow_small_or_imprecise_dtypes=True,
            )
            ones_bf = const_pool.tile([128, 1], BF16, name="ones_bf")
            nc.vector.memset(ones_bf[:], 1.0)

            # ---- DMA schedule ----
            # stream order: tx0, attrs, b7 tiles 0..5 (pairs), tx1..tx4,
            # b5 pairs, b6 pairs, b7 tile 6, b7 tile 7.  One SBUF tile PER
            # PAIR DMA for the tails (dependency tracking can be
            # tile-granular).  `arrive` mirrors the cost model's serial
            # transfer schedule so the Tile scheduler can be given true
            # ready times (its own DMA model mispredicts, which otherwise
            # reorders the tail queues).
            HEAD_NS = 1966
            TX_NS, ATTR_NS, PAIR_NS, SNG_NS = 3004, 205, 728, 364
            arrival = {}
            _cursor = [HEAD_NS]

            def arrive(key, dur):
                _cursor[0] += dur
                arrival[key] = _cursor[0]

            txs = [None] * BPC
            txp = {b: [None] * (NDT // 2 + 2) for b in TAIL}

            def tail_pair_dma(b, tp):
                tp_t = text_pool.tile(
                    [128, 2 * TILE_B], F8, tag=f"tx{b}", bufs=NDT // 2 + 2,
                    name=f"tx_{b}_{tp}",
                )
                txp[b][tp] = tp_t
                nc.sync.dma_start(
                    tp_t[:], text_d[b, :, tp * 2 * TILE_B:(tp + 1) * 2 * TILE_B]
                )
                arrive(("p", b, tp), PAIR_NS)

            txs[0] = text_pool.tile([128, ROW_B], F8, tag="tx", name="tx_0")
            nc.sync.dma_start(txs[0][:], text_d[0])
            arrival[("f", 0)] = HEAD_NS + TX_NS
            attr_sb = const_pool.tile([128, ATTR_W], I8, name="attr_sb")
            nc.sync.dma_start(attr_sb[:], attrs_d[:])
            _cursor[0] = HEAD_NS + TX_NS
            arrive("attr", ATTR_NS)
            # b7 d-tile pairs 0..2 arrive early (tile-major packing)
            for tp in range(3):
                tail_pair_dma(b7, tp)
            for b in range(1, NFULL):
                txs[b] = text_pool.tile([128, ROW_B], F8, tag="tx", name=f"tx_{b}")
                nc.sync.dma_start(txs[b][:], text_d[b])
                arrive(("f", b), TX_NS)
            for b in (b5, b6):
                for tp in range(NDT // 2):
                    tail_pair_dma(b, tp)
            # the closing single-tile DMAs of sample 7
            for k, t in enumerate((NDT - 2, NDT - 1)):
                st = text_pool.tile(
                    [128, TILE_B], F8, tag=f"tx{b7}", bufs=NDT // 2 + 2,
                    name=f"tx_{b7}_s{t}",
                )
                txp[b7][NDT // 2 + k] = st
                nc.sync.dma_start(
                    st[:], text_d[b7, :, t * TILE_B:(t + 1) * TILE_B]
                )
                arrive(("s", b7, t), SNG_NS)

            def at_ns(ns):
                # schedule hint: treat wrapped instructions as becoming
                # ready at `ns` (DMA-arrival-derived) in the scheduler
                return tc.tile_wait_until(ns / 1e6)

            # per-sample results: (num | nss), finished on the host;
            # raw ss planes for the last six d-tiles to land
            asm = const_pool.tile([1, BPC, 2 * S], F32, name="asm")
            raw = const_pool.tile([128, NRAW, S], BF16, name="raw")

            combos = []
            for b in range(BPC):
                cb = combo_pool.tile([128, NDT, 2 * S], BF16, tag="cb", name=f"cb_{b}")
                combos.append(cb)

            def vg_view(b):
                if b in TAIL:
                    i = b - TAIL[0]
                    lo = BPC * NCHUNK + VGT_OFF[i]
                    return attr_sb[:, lo:lo + VGT_TILES[i] * S].bitcast(
                        F8
                    ).rearrange("p (t s) -> p t s", s=S)
                return txs[b][:, TXT_B:ROW_B].rearrange("p (t s) -> p t s", s=S)

            def onehot(b, tag="oh", bufs=None):
                kw = {} if bufs is None else {"bufs": bufs}
                oh_all = oh_pool.tile([128, NCHUNK * S], F8, tag=tag,
                                      name=f"oh_{b}", **kw)
                nc.vector.tensor_tensor(
                    oh_all[:].rearrange("p (c s) -> p c s", s=S),
                    attr_sb[:, b * NCHUNK:(b + 1) * NCHUNK]
                    .unsqueeze(2).broadcast_to([128, NCHUNK, S]),
                    iota_s[:].unsqueeze(1).broadcast_to([128, NCHUNK, S]),
                    op=ALU.is_equal,
                )
                return oh_all[:].rearrange("p (c s) -> p c s", s=S)

            def full_sample(b):
                tx = txs[b]
                vg_v = vg_view(b)
                cb = combos[b]
                # combo[b]: [128, t, (prod | ss^2)]
                oh_v = onehot(b)

                # pair-major packing: [p, c, i, d]; d-tile outer so each
                # PSUM chain is 4 consecutive matmuls and drains at once
                tx_v = tx[:, 0:TXT_B].rearrange(
                    "p (c i d) -> p c i d", c=NPAIR, i=2
                )
                sst = sst_pool.tile([128, NDT, S], BF16, tag="sst", name=f"sst_{b}")
                for t in range(NDT):
                    pst = psum_pool.tile(
                        [128, S], F32, tag="ss", name=f"pst_{b}_{t}"
                    )
                    for c in range(NPAIR):
                        nc.tensor.matmul(
                            pst[:],
                            tx_v[:, c, :, t * 128:(t + 1) * 128],
                            oh_v[:, 2 * c:2 * c + 2, :],
                            start=(c == 0), stop=(c == NPAIR - 1),
                            perf_mode=PERF.DoubleRow,
                        )
                    # drain the bank (split the copies between DVE and ACT)
                    if t % 2 == 0:
                        nc.vector.tensor_copy(sst[:, t, :], pst[:])
                    else:
                        nc.scalar.activation(sst[:, t, :], pst[:], ACTF.Copy)
                # batched [128, 256] prod and ss^2 on DVE (2x for bf16)
                nc.vector.tensor_tensor(cb[:, :, 0:S], sst[:], vg_v, op=ALU.mult)
                nc.scalar.activation(cb[:, :, S:2 * S], sst[:], ACTF.Square)

                # partition-reduce (num | ss^2) over d via ones-matmul
                red = psumr_pool.tile([1, 2 * S], F32, tag="red", name=f"red_{b}")
                for t in range(NDT):
                    nc.tensor.matmul(
                        red[:], ones_bf[:], cb[:, t, :],
                        start=(t == 0), stop=(t == NDT - 1),
                    )
                if b % 2 == 0:
                    nc.vector.tensor_copy(asm[:, b, :], red[:])
                else:
                    nc.scalar.activation(asm[:, b, :], red[:], ACTF.Copy)

            # ---- tail helpers ----
            def tail_tile_mm(src, t2, b):
                src_v = src[:].rearrange(
                    "p (t c i e) -> p t c i e", t=src.shape[1] // TILE_B,
                    c=NPAIR, i=2,
                )
                pst = psum_pool.tile([128, S], F32, tag="ss", name=f"pst{b}_{t2}")
                for c in range(NPAIR):
                    nc.tensor.matmul(
                        pst[:],
                        src_v[:, t2, c, :, :],
                        oh_tail[b][:, 2 * c:2 * c + 2, :],
                        start=(c == 0), stop=(c == NPAIR - 1),
                        perf_mode=PERF.DoubleRow,
                    )
                return pst

            def tail_pair(b, tp):
                # drain the pair's banks (one ACT copy, one DVE copy), then
                # batched pair prod and square on DVE from SBUF (PSUM-direct
                # squaring is illegal - only one PSUM input per op)
                sst = sst_pool.tile(
                    [128, 2, S], BF16, tag=f"sstt{b}", bufs=NDT // 2,
                    name=f"sstt_{b}_{tp}",
                )
                for t2 in range(2):
                    pst = tail_tile_mm(txp[b][tp], t2, b)
                    if t2 == 0:
                        nc.scalar.activation(sst[:, t2, :], pst[:], ACTF.Copy)
                    else:
                        nc.vector.tensor_copy(sst[:, t2, :], pst[:])
                tsl = slice(2 * tp, 2 * tp + 2)
                nc.vector.tensor_tensor(
                    combos[b][:, tsl, 0:S], sst[:],
                    vg_view(b)[:, tsl, :], op=ALU.mult,
                )
                nc.vector.tensor_tensor(
                    combos[b][:, tsl, S:2 * S], sst[:], sst[:], op=ALU.mult
                )

            def raw_tile(src, t2, b, k, dve):
                # critical-path tile: segment-sum plane copied PSUM->SBUF
                # in ONE op and shipped raw; the host does num/nss for it
                pst = tail_tile_mm(src, t2, b)
                if dve:
                    nc.vector.tensor_copy(raw[:, k, :], pst[:])
                else:
                    nc.scalar.activation(raw[:, k, :], pst[:], ACTF.Copy)

            # ---- emission in data-arrival order, with schedule hints ----
            with at_ns(arrival[("f", 0)] + 900):
                full_sample(0)

            # tail early work: onehots
            with at_ns(arrival["attr"] + 900):
                oh_tail = {
                    b: onehot(b, tag=f"oht{b}", bufs=1) for b in TAIL
                }

            # b7 tiles 0..5 (early pairs) and its reduce, closed at t5
            for tp in range(3):
                with at_ns(arrival[("p", b7, tp)] + 900):
                    tail_pair(b7, tp)
            with at_ns(arrival[("p", b7, 2)] + 2600):
                red7 = psumr7_pool.tile([1, 2 * S], F32, tag="red7", name="red7")
                for t in range(NDT - 2):
                    nc.tensor.matmul(
                        red7[:], ones_bf[:], combos[b7][:, t, 0:2 * S],
                        start=(t == 0), stop=(t == NDT - 3),
                    )
                nc.vector.tensor_copy(asm[:, b7, 0:2 * S], red7[:])

            for b in range(1, NFULL):
                with at_ns(arrival[("f", b)] + 900):
                    full_sample(b)

            # b5 pairs + full reduce
            for tp in range(NDT // 2):
                with at_ns(arrival[("p", b5, tp)] + 900):
                    tail_pair(b5, tp)
            with at_ns(arrival[("p", b5, 3)] + 2600):
                red5 = psumr_pool.tile([1, 2 * S], F32, tag="red", name="red5")
                for t in range(NDT):
                    nc.tensor.matmul(
                        red5[:], ones_bf[:], combos[b5][:, t, 0:2 * S],
                        start=(t == 0), stop=(t == NDT - 1),
                    )
                nc.scalar.activation(asm[:, b5, 0:2 * S], red5[:], ACTF.Copy)

            # b6 pair 0; its reduce closes at t1 (tiles 2..7 go raw)
            with at_ns(arrival[("p", b6, 0)] + 900):
                tail_pair(b6, 0)
            with at_ns(arrival[("p", b6, 0)] + 2600):
                red6 = psumr_pool.tile([1, 2 * S], F32, tag="red", name="red6")
                for t in range(2):
                    nc.tensor.matmul(
                        red6[:], ones_bf[:], combos[b6][:, t, 0:2 * S],
                        start=(t == 0), stop=(t == 1),
                    )
                nc.scalar.activation(asm[:, b6, 0:2 * S], red6[:], ACTF.Copy)

            # (num | nss) block: gated only on mid-stream closes
            with at_ns(arrival[("p", b6, 0)] + 3300):
                nc.sync.dma_start(out_d[:], asm[:].rearrange("o b s -> o (b s)"))

            # ---- the tail: eight raw ss planes, then the raw DMA ----
            for tp in (1, 2, 3):
                with at_ns(arrival[("p", b6, tp)] + 900):
                    raw_tile(txp[b6][tp], 0, b6, 2 * (tp - 1), dve=True)
                    raw_tile(txp[b6][tp], 1, b6, 2 * (tp - 1) + 1, dve=False)
            # planes 0..5 (b6) go out as soon as they are ready so their
            # HWDGE descriptor-gen runs under the b7 tail chain; the closing
            # DMA carries only b7's two planes (91 ns transfer)
            with at_ns(arrival[("p", b6, 3)] + 1700):
                nc.sync.dma_start(
                    raw_d[:, 0:6 * S], raw[:, 0:6, :].rearrange("p k s -> p (k s)")
                )
            with at_ns(arrival[("s", b7, NDT - 2)] + 900):
                raw_tile(txp[b7][NDT // 2], 0, b7, 6, dve=False)
            with at_ns(arrival[("s", b7, NDT - 1)] + 900):
                raw_tile(txp[b7][NDT // 2 + 1], 0, b7, 7, dve=True)
            with at_ns(arrival[("s", b7, NDT - 1)] + 1700):
                nc.sync.dma_start(
                    raw_d[:, 6 * S:8 * S], raw[:, 6:8, :].rearrange("p k s -> p (k s)")
                )

    nc.compile()
    return nc


def pack_shard(attributes, text_feats, Vgs):
    """Host-side packing of one core's shard into the kernel's dram layout."""
    at = np.asarray(attributes)
    # attr[p, b, c] = attributes[b, c*128 + p], int8 (values 0..32),
    # followed by the tail samples' transposed Vgs blocks (fp8 bytes)
    attr_tp = np.empty((128, ATTR_W), dtype=np.int8)
    attr_tp[:, 0:BPC * NCHUNK] = (
        at.reshape(BPC, NCHUNK, 128).transpose(2, 0, 1)
        .reshape(128, BPC * NCHUNK).astype(np.int8)
    )

    tf8 = np.asarray(text_feats, dtype=np.float32).astype(NP_F8)
    vg8 = np.asarray(Vgs, dtype=np.float32).astype(NP_F8)
    t8 = np.empty((BPC, 128, ROW_B), dtype=NP_F8)
    x = tf8.reshape(BPC, NPAIR, 2, 128, D)
    for b in range(NFULL):
        # [p, c, i, d]
        t8[b, :, 0:TXT_B] = x[b].transpose(2, 0, 1, 3).reshape(128, TXT_B)
    for b in range(NFULL, BPC):
        # tail samples: d-tile-major [p, t, c, i, e]
        xb = x[b].reshape(NPAIR, 2, 128, NDT, 128)
        t8[b, :, 0:TXT_B] = xb.transpose(2, 3, 0, 1, 4).reshape(128, TXT_B)
    # vgt: [p, t, s] = Vgs[b, s, t*128+p]; fulls inline, tails via attrs
    # (tails carry only the d-tiles whose prod runs on device)
    vgt = vg8.reshape(BPC, S, NDT, 128).transpose(0, 3, 2, 1)
    t8[:, :, TXT_B:ROW_B] = vgt.reshape(BPC, 128, NDT * S)
    for i, b in enumerate(range(NFULL, BPC)):
        lo = BPC * NCHUNK + VGT_OFF[i]
        nt = VGT_TILES[i]
        attr_tp[:, lo:lo + nt * S] = (
            vgt[b, :, 0:nt, :].reshape(128, nt * S).view(np.int8)
        )
    return {"attributes": attr_tp, "text_feats": t8}


_NC_CACHE = None


def _get_nc():
    global _NC_CACHE
    if _NC_CACHE is None:
        _NC_CACHE = build_bass()
    return _NC_CACHE


# raw plane k -> (sample index, d-tile)
RAW_MAP = [(BPC - 2, 2), (BPC - 2, 3), (BPC - 2, 4), (BPC - 2, 5),
           (BPC - 2, 6), (BPC - 2, 7), (BPC - 1, 6), (BPC - 1, 7)]


def _finish(out_flat, raw_flat, Vgs_shard):
    """Host finish for one core: fold the raw ss planes into (num, nss),
    compute nvg from f32 Vgs, then cos = num / sqrt(nss * nvg), summed."""
    arr = np.asarray(out_flat, dtype=np.float64).reshape(BPC, 2, S)
    num, nss = arr[:, 0, :].copy(), arr[:, 1, :].copy()
    vg = np.asarray(Vgs_shard, dtype=np.float64)       # [BPC, S, D]
    rw = np.asarray(raw_flat, dtype=np.float64).reshape(128, NRAW, S)
    for k, (b, t) in enumerate(RAW_MAP):
        ss = rw[:, k, :]                               # [128p, S] = ss[d,s]
        vg_t = vg[b, :, t * 128:(t + 1) * 128].T       # [128p, S]
        num[b] += (ss * vg_t).sum(axis=0)
        nss[b] += (ss * ss).sum(axis=0)
    nvg = (vg * vg).sum(axis=2)                        # [BPC, S]
    den = np.maximum(np.sqrt(nss * nvg), EPS)
    return float((num / den).sum())


def kernel(attributes: np.ndarray, text_feats: np.ndarray, Vgs: np.ndarray) -> np.ndarray:
    assert attributes.shape == (B, L) and attributes.dtype == np.int32
    assert text_feats.shape == (B, L, D)
    assert Vgs.shape == (B, S, D)
    nc = _get_nc()
    in_maps = [
        pack_shard(
            attributes[i * BPC:(i + 1) * BPC],
            text_feats[i * BPC:(i + 1) * BPC],
            Vgs[i * BPC:(i + 1) * BPC],
        )
        for i in range(N_CORES)
    ]
    res = run_bass_kernel_spmd(nc, in_maps, core_ids=list(range(N_CORES)))
    total = sum(
        _finish(r["out"], r["raw"], Vgs[i * BPC:(i + 1) * BPC])
        for i, r in enumerate(res.results)
    )
    loss = 1.0 - total / (B * S)
    return np.asarray(loss, dtype=np.float32)
